# revision 17
# baseline (speedup 1.0000x reference)
"""Soft decision-tree forward kernel for Trainium2 (8 NeuronCores, SPMD).

Per core (16384 rows), fp16 data path, f32 accumulation:
  1. z06 = [x|1]^T-tile @ G06          (PE, per 128-row tile, K=33, N=128)
  2. act06 = sigmoid(z06)              (ACT, one op per 8 tiles, fp16 out)
  3. tree DP levels 1..6, batch-major  (DVE, multi-AP ops over 32-tile groups)
  4. P7 -> DRAM -> xbar-transpose back (DMA) giving node-major P7T [128, B]
  5. z7T = G7 @ xq, sig7T = sigmoid(z7T - T7)   (PE K=32 + ACT bias, early)
  6. R7T = P7T * sig7T                 (DVE, node-major)
  7. pT = A^T@P7T + B^T@R7T            (PE, col-tiled 4x, PSUM accumulate)
  8. pT -> SBUF -> DRAM [10, 16384]    (host un-permutes columns)

Node layout: level d's column k <-> heap node bitrev_d(k) (concat child
placement). All reorderings are baked into G06 / G7 / A / B host-side.
P7 DRAM row order is q = (s, p, g): s = supertile, p = row-in-tile,
g = tile-in-supertile; batch b = s*4096 + g*128 + p. xQ input carries x
columns pre-permuted to q order so node-major matmul reads are contiguous.
"""

import sys

sys.path.insert(0, "/opt/trn_rl_repo")

import numpy as np

import concourse.bacc as bacc
import concourse.bass as bass
import concourse.mybir as mybir
import concourse.tile as tile
from concourse import bass_utils

# ---- problem constants (hardcoded per contract) ----
BATCH = 131072
N_FEAT = 32
N_CLASSES = 10
N_CORES = 8
R = BATCH // N_CORES          # 16384 rows per core
TILE = 128
N_TILES = R // TILE           # 128
G_TILES = 32                  # tiles per supertile (DVE grouping)
N_SUPER = N_TILES // G_TILES  # 4
SUPER_ROWS = G_TILES * TILE   # 4096
KDIM = N_FEAT + 1             # 33
PSUM_TILES = 8                # z06 tiles per PSUM buffer -> ACT op of N=1024
NM_CHUNK = 1024               # node-major batch chunk (z7T / sig7T / R7T)
N_CHUNKS = R // NM_CHUNK      # 16
FIN_SUB = 512                 # final matmul sub-chunk (one PSUM bank)

F32 = mybir.dt.float32
F16 = mybir.dt.float16
SIGMOID = mybir.ActivationFunctionType.Sigmoid

_COMPILED = None


def _bitrev(k, bits):
    r = 0
    for _ in range(bits):
        r = (r << 1) | (k & 1)
        k >>= 1
    return r


def _host_prep(thresholds, feats, leaf_class):
    """G06 [33,128], G7 [32,128], negT7 [128,1], A/B [128,10] (device layout)."""
    G06 = np.zeros((KDIM, 128), dtype=np.float32)
    f0, t0 = int(feats[0]), float(thresholds[0])
    G06[f0, 0] = -1.0
    G06[N_FEAT, 0] = +t0
    G06[f0, 1] = +1.0
    G06[N_FEAT, 1] = -t0
    for d in range(1, 7):
        n = 1 << d
        start = n - 1
        for k in range(n):
            j = _bitrev(k, d)
            G06[int(feats[start + j]), n + k] = 1.0
            G06[N_FEAT, n + k] = -float(thresholds[start + j])
    G7 = np.zeros((N_FEAT, 128), dtype=np.float32)
    negT7 = np.zeros((128, 1), dtype=np.float32)
    start7 = 127
    for k in range(128):
        j = _bitrev(k, 7)
        G7[int(feats[start7 + j]), k] = 1.0
        negT7[k, 0] = -float(thresholds[start7 + j])
    Lc = np.empty(128, dtype=np.int64)
    Rc = np.empty(128, dtype=np.int64)
    for k in range(128):
        j7 = _bitrev(k, 7)
        Lc[k] = leaf_class[2 * j7]
        Rc[k] = leaf_class[2 * j7 + 1]
    A = np.zeros((128, N_CLASSES), dtype=np.float32)
    Bm = np.zeros((128, N_CLASSES), dtype=np.float32)
    A[np.arange(128), Lc] = 1.0
    Bm[np.arange(128), Rc] += 1.0
    Bm[np.arange(128), Lc] -= 1.0
    return G06, G7, negT7, A, Bm


def _build_program():
    nc = bacc.Bacc("TRN2", target_bir_lowering=False, debug=False,
                   num_devices=N_CORES)

    xT_d = nc.dram_tensor("xT", [KDIM, R], F16, kind="ExternalInput")
    xq_d = nc.dram_tensor("xQ", [N_FEAT, R], F16, kind="ExternalInput")
    g06_d = nc.dram_tensor("G06", [KDIM, 128], F16, kind="ExternalInput")
    g7_d = nc.dram_tensor("G7", [N_FEAT, 128], F16, kind="ExternalInput")
    negT7_d = nc.dram_tensor("negT7", [128, 1], F32, kind="ExternalInput")
    a_d = nc.dram_tensor("A", [128, N_CLASSES], F16, kind="ExternalInput")
    b_d = nc.dram_tensor("B", [128, N_CLASSES], F16, kind="ExternalInput")
    pt_d = nc.dram_tensor("pT", [N_CLASSES, R], F32, kind="ExternalOutput")

    with tile.TileContext(nc) as tc:
        with (
            tc.tile_pool(name="const", bufs=1) as cpool,
            tc.tile_pool(name="dram", bufs=1, space="DRAM") as dpool,
            tc.tile_pool(name="act06", bufs=2) as act_pool,
            tc.tile_pool(name="ptree", bufs=1) as tree_pool,
            tc.tile_pool(name="p7", bufs=2) as p7_pool,
            tc.tile_pool(name="p7t", bufs=2) as p7t_pool,
            tc.tile_pool(name="sig", bufs=1) as sig_pool,
            tc.tile_pool(name="nm", bufs=2) as nm_pool,
            tc.tile_pool(name="ptout", bufs=1) as pt_pool,
            tc.tile_pool(name="zpsum", bufs=2, space="PSUM") as zpsum,
            tc.tile_pool(name="z7psum", bufs=2, space="PSUM") as z7psum,
        ):
            xT = cpool.tile([KDIM, R], F16, tag="xT")
            nc.sync.dma_start(xT[:], xT_d.ap()[:, :])
            xq = cpool.tile([N_FEAT, R], F16, tag="xQ")
            nc.sync.dma_start(xq[:], xq_d.ap()[:, :])
            g06 = cpool.tile([KDIM, 128], F16, tag="G06")
            nc.sync.dma_start(g06[:], g06_d.ap()[:, :])
            g7 = cpool.tile([N_FEAT, 128], F16, tag="G7")
            nc.sync.dma_start(g7[:], g7_d.ap()[:, :])
            negT7 = cpool.tile([128, 1], F32, tag="negT7")
            nc.sync.dma_start(negT7[:], negT7_d.ap()[:, :])
            a_s = cpool.tile([128, N_CLASSES], F16, tag="A")
            nc.sync.dma_start(a_s[:], a_d.ap()[:, :])
            b_s = cpool.tile([128, N_CLASSES], F16, tag="B")
            nc.sync.dma_start(b_s[:], b_d.ap()[:, :])

            # P7 staging in DRAM, row order q = (s, p, g)
            p7dram = dpool.tile([R, 128], F16, tag="p7dram")
            p7dram_v = p7dram[:].rearrange(
                "(s p g) j -> s p g j", s=N_SUPER, p=TILE, g=G_TILES)

            pt_out = pt_pool.tile([128, 8 * FIN_SUB], F32, tag="ptout")

            # --- early node-major phase: z7T + sig7T for all chunks ---
            sig7s = []
            for ch in range(N_CHUNKS):
                z7t = z7psum.tile([128, NM_CHUNK], F32, tag="z7T", name="z7t")
                for hf in range(NM_CHUNK // FIN_SUB):
                    rhs = xq[:, ch * NM_CHUNK + hf * FIN_SUB:
                             ch * NM_CHUNK + (hf + 1) * FIN_SUB]
                    nc.tensor.matmul(z7t[:, bass.ts(hf, FIN_SUB)],
                                     g7[:], rhs, start=True, stop=True)
                sig7 = sig_pool.tile([128, NM_CHUNK], F16, tag=f"sig{ch}",
                                     name=f"sig{ch}")
                nc.scalar.activation(sig7[:], z7t[:], SIGMOID, bias=negT7[:])
                sig7s.append(sig7)

            fin_state = {}

            for s in range(N_SUPER):
                act06 = act_pool.tile([TILE, G_TILES, 128], F16, tag="act06")
                # --- z06 matmuls + sigmoid, 8 tiles per PSUM buffer ---
                for pb in range(G_TILES // PSUM_TILES):
                    z06 = zpsum.tile([TILE, PSUM_TILES * 128], F32, tag="z06",
                                     name="z06")
                    for i in range(PSUM_TILES):
                        t = s * G_TILES + pb * PSUM_TILES + i
                        nc.tensor.matmul(
                            z06[:, bass.ts(i, 128)],
                            xT[:, bass.ts(t, TILE)], g06[:],
                            start=True, stop=True,
                        )
                    nc.scalar.activation(
                        act06[:, bass.ts(pb, PSUM_TILES), :], z06[:], SIGMOID)
                # --- tree DP levels 1..6 (batch-major, grouped) ---
                prev = act06[:, :, 0:2]
                for d in range(1, 7):
                    n = 1 << d
                    if d < 6:
                        cur = tree_pool.tile([TILE, G_TILES, 2 * n], F16,
                                             tag=f"P{d + 1}", name=f"P{d + 1}")
                    else:
                        cur = p7_pool.tile([TILE, G_TILES, 128], F16,
                                           tag="P7", name="P7")
                    nc.vector.tensor_mul(
                        cur[:, :, n:2 * n], prev[:], act06[:, :, n:2 * n])
                    nc.vector.tensor_sub(
                        cur[:, :, 0:n], prev[:], cur[:, :, n:2 * n])
                    prev = cur[:, :, :]
                nc.sync.dma_start(p7dram_v[s], prev[:])

                # --- transpose this supertile back, node-major ---
                p7t = p7t_pool.tile([128, SUPER_ROWS], F16, tag="P7T",
                                    name="p7t")
                nc.sync.dma_start_transpose(
                    p7t[:], p7dram[bass.ts(s, SUPER_ROWS), :])

                for cc in range(SUPER_ROWS // NM_CHUNK):
                    ch = s * (SUPER_ROWS // NM_CHUNK) + cc
                    r7t = nm_pool.tile([128, NM_CHUNK], F16, tag="R7T",
                                       name="r7t")
                    p7t_sl = p7t[:, bass.ts(cc, NM_CHUNK)]
                    nc.vector.tensor_mul(r7t[:], p7t_sl, sig7s[ch][:])
                    # finals: col-group cg cycles 0..3; 4 subs share a psum
                    for half in range(NM_CHUNK // FIN_SUB):
                        u = ch * (NM_CHUNK // FIN_SUB) + half  # 0..31
                        k, cg = divmod(u, 4)
                        if cg == 0:
                            fin_state["fp"] = zpsum.tile(
                                [128, FIN_SUB], F32, tag="z06", name="fin")
                        fp = fin_state["fp"]
                        out_sl = fp[32 * cg:32 * cg + N_CLASSES, :]
                        rhs_p = p7t[:, cc * NM_CHUNK + half * FIN_SUB:
                                    cc * NM_CHUNK + (half + 1) * FIN_SUB]
                        rhs_r = r7t[:, bass.ts(half, FIN_SUB)]
                        nc.tensor.matmul(out_sl, a_s[:], rhs_p,
                                         start=True, stop=False,
                                         tile_position=(0, 32 * cg))
                        nc.tensor.matmul(out_sl, b_s[:], rhs_r,
                                         start=False, stop=True,
                                         tile_position=(0, 32 * cg))
                        if cg == 3:
                            nc.vector.tensor_copy(
                                pt_out[:, bass.ts(k, FIN_SUB)],
                                fp[:, 0:FIN_SUB])

            # --- output DMA: 4 strided DMAs, one per col-group ---
            # pt_out[32*cg + c, k*512 + scol] = pT[c, q], q = (4k+cg)*512+scol
            pt_v = pt_d.ap().rearrange("c (k cg scol) -> c k cg scol",
                                       k=8, cg=4, scol=FIN_SUB)
            for cg in range(4):
                src = pt_out[32 * cg:32 * cg + N_CLASSES, :].rearrange(
                    "c (k scol) -> c k scol", k=8, scol=FIN_SUB)
                nc.sync.dma_start(pt_v[:, :, cg, :], src)

    nc.compile()
    return nc


def _get_compiled():
    global _COMPILED
    if _COMPILED is None:
        _COMPILED = _build_program()
    return _COMPILED


def kernel(x, thresholds, feats, leaf_class, _trace=False):
    x = np.asarray(x, dtype=np.float32)
    thresholds = np.asarray(thresholds, dtype=np.float32)
    feats = np.asarray(feats, dtype=np.int32)
    leaf_class = np.asarray(leaf_class, dtype=np.int32)
    assert x.shape == (BATCH, N_FEAT)

    G06, G7, negT7, A, Bm = _host_prep(thresholds, feats, leaf_class)
    f16 = np.float16

    x_ext_T = np.empty((KDIM, BATCH), dtype=f16)
    x_ext_T[:N_FEAT, :] = x.T.astype(f16)
    x_ext_T[N_FEAT, :] = 1.0

    in_maps = []
    for c in range(N_CORES):
        sl = slice(c * R, (c + 1) * R)
        xt_c = np.ascontiguousarray(x_ext_T[:, sl])
        # q-order: q = s*4096 + p*32 + g  <->  b = s*4096 + g*128 + p
        xq_c = np.ascontiguousarray(
            xt_c[:N_FEAT].reshape(N_FEAT, N_SUPER, G_TILES, TILE)
            .transpose(0, 1, 3, 2).reshape(N_FEAT, R))
        in_maps.append({
            "xT": xt_c,
            "xQ": xq_c,
            "G06": G06.astype(f16),
            "G7": G7.astype(f16),
            "negT7": negT7,
            "A": A.astype(f16),
            "B": Bm.astype(f16),
        })

    nc = _get_compiled()
    res = bass_utils.run_bass_kernel_spmd(
        nc, in_maps, core_ids=list(range(N_CORES)),
        trace=_trace, trace_cores=[0] if _trace else None,
    )

    out = np.empty((BATCH, N_CLASSES), dtype=np.float32)
    for c in range(N_CORES):
        sl = slice(c * R, (c + 1) * R)
        pt = res.results[c]["pT"]  # [10, R], cols in q=(s,p,g) order
        # q = s*4096 + p*32 + g ; b = s*4096 + g*128 + p
        pt_b = pt.reshape(N_CLASSES, N_SUPER, TILE, G_TILES) \
                 .transpose(0, 1, 3, 2).reshape(N_CLASSES, R)
        out[sl] = pt_b.T
    if _trace:
        kernel._last_results = res
    return out


# revision 21
# speedup vs baseline: 1.5976x; 1.5976x over previous
"""Soft decision-tree forward kernel for Trainium2 (8 NeuronCores, SPMD).

Per core (16384 rows), fp16 data path, f32 accumulation:
  1. z06 = [x|1]^T-tile @ G06          (PE, per 128-row tile, K=33, N=128)
  2. act06 = sigmoid(z06)              (ACT, one op per 8 tiles, fp16 out)
  3. tree DP levels 1..6, batch-major  (DVE, multi-AP ops over 32-tile groups)
  4. P7 -> DRAM -> xbar-transpose back (DMA) giving node-major P7T [128, B]
  5. z7T = G7 @ xq, sig7T = sigmoid(z7T - T7)   (PE K=32 + ACT bias, early)
  6. R7T = P7T * sig7T                 (DVE, node-major)
  7. pT = A^T@P7T + B^T@R7T            (PE, col-tiled 4x, PSUM accumulate)
  8. pT -> SBUF -> DRAM [10, 16384]    (host un-permutes columns)

Node layout: level d's column k <-> heap node bitrev_d(k) (concat child
placement). All reorderings are baked into G06 / G7 / A / B host-side.
P7 DRAM row order is q = (s, p, g): s = supertile, p = row-in-tile,
g = tile-in-supertile; batch b = s*4096 + g*128 + p. xQ input carries x
columns pre-permuted to q order so node-major matmul reads are contiguous.
"""

import sys

sys.path.insert(0, "/opt/trn_rl_repo")

import numpy as np

import concourse.bacc as bacc
import concourse.bass as bass
import concourse.mybir as mybir
import concourse.tile as tile
from concourse import bass_utils

# ---- problem constants (hardcoded per contract) ----
BATCH = 131072
N_FEAT = 32
N_CLASSES = 10
N_CORES = 8
R = BATCH // N_CORES          # 16384 rows per core
TILE = 128
N_TILES = R // TILE           # 128
G_TILES = 32                  # tiles per supertile (DVE grouping)
N_SUPER = N_TILES // G_TILES  # 4
SUPER_ROWS = G_TILES * TILE   # 4096
KDIM = N_FEAT + 1             # 33
PSUM_TILES = 8                # z06 tiles per PSUM buffer -> ACT op of N=1024
NM_CHUNK = 1024               # node-major batch chunk (z7T / sig7T / R7T)
N_CHUNKS = R // NM_CHUNK      # 16
FIN_SUB = 512                 # final matmul sub-chunk (one PSUM bank)

F32 = mybir.dt.float32
F16 = mybir.dt.float16
SIGMOID = mybir.ActivationFunctionType.Sigmoid

_COMPILED = None


def _bitrev(k, bits):
    r = 0
    for _ in range(bits):
        r = (r << 1) | (k & 1)
        k >>= 1
    return r


def _host_prep(thresholds, feats, leaf_class):
    """G06 [33,128], G7 [32,128], negT7 [128,1], A/B [128,10] (device layout)."""
    G06 = np.zeros((KDIM, 128), dtype=np.float32)
    f0, t0 = int(feats[0]), float(thresholds[0])
    G06[f0, 0] = -1.0
    G06[N_FEAT, 0] = +t0
    G06[f0, 1] = +1.0
    G06[N_FEAT, 1] = -t0
    for d in range(1, 7):
        n = 1 << d
        start = n - 1
        for k in range(n):
            j = _bitrev(k, d)
            G06[int(feats[start + j]), n + k] = 1.0
            G06[N_FEAT, n + k] = -float(thresholds[start + j])
    G7 = np.zeros((N_FEAT, 128), dtype=np.float32)
    negT7 = np.zeros((128, 1), dtype=np.float32)
    start7 = 127
    for k in range(128):
        j = _bitrev(k, 7)
        G7[int(feats[start7 + j]), k] = 1.0
        negT7[k, 0] = -float(thresholds[start7 + j])
    Lc = np.empty(128, dtype=np.int64)
    Rc = np.empty(128, dtype=np.int64)
    for k in range(128):
        j7 = _bitrev(k, 7)
        Lc[k] = leaf_class[2 * j7]
        Rc[k] = leaf_class[2 * j7 + 1]
    A = np.zeros((128, N_CLASSES), dtype=np.float32)
    Bm = np.zeros((128, N_CLASSES), dtype=np.float32)
    A[np.arange(128), Lc] = 1.0
    Bm[np.arange(128), Rc] += 1.0
    Bm[np.arange(128), Lc] -= 1.0
    return G06, G7, negT7, A, Bm


def _build_program():
    nc = bacc.Bacc("TRN2", target_bir_lowering=False, debug=False,
                   num_devices=N_CORES)

    xT_d = nc.dram_tensor("xT", [KDIM, R], F16, kind="ExternalInput")
    xq_d = nc.dram_tensor("xQ", [N_FEAT, R], F16, kind="ExternalInput")
    g06_d = nc.dram_tensor("G06", [KDIM, 128], F16, kind="ExternalInput")
    g7_d = nc.dram_tensor("G7", [N_FEAT, 128], F16, kind="ExternalInput")
    negT7_d = nc.dram_tensor("negT7", [128, 1], F32, kind="ExternalInput")
    a_d = nc.dram_tensor("A", [128, N_CLASSES], F16, kind="ExternalInput")
    b_d = nc.dram_tensor("B", [128, N_CLASSES], F16, kind="ExternalInput")
    pt_d = nc.dram_tensor("pT", [N_CLASSES, R], F32, kind="ExternalOutput")

    with tile.TileContext(nc) as tc:
        with (
            tc.tile_pool(name="const", bufs=1) as cpool,
            tc.tile_pool(name="dram", bufs=1, space="DRAM") as dpool,
            tc.tile_pool(name="act06", bufs=2) as act_pool,
            tc.tile_pool(name="ptree", bufs=1) as tree_pool,
            tc.tile_pool(name="p7", bufs=2) as p7_pool,
            tc.tile_pool(name="p7t", bufs=2) as p7t_pool,
            tc.tile_pool(name="sig", bufs=1) as sig_pool,
            tc.tile_pool(name="nm", bufs=2) as nm_pool,
            tc.tile_pool(name="ptout", bufs=1) as pt_pool,
            tc.tile_pool(name="zpsum", bufs=2, space="PSUM") as zpsum,
            tc.tile_pool(name="z7psum", bufs=1, space="PSUM") as z7psum,
            tc.tile_pool(name="fpsum", bufs=2, space="PSUM") as fpsum,
        ):
            xT = cpool.tile([KDIM, R], F16, tag="xT")
            nc.sync.dma_start(xT[:], xT_d.ap()[:, :])
            xq = cpool.tile([N_FEAT, R], F16, tag="xQ")
            nc.sync.dma_start(xq[:], xq_d.ap()[:, :])
            g06 = cpool.tile([KDIM, 128], F16, tag="G06")
            nc.sync.dma_start(g06[:], g06_d.ap()[:, :])
            g7 = cpool.tile([N_FEAT, 128], F16, tag="G7")
            nc.sync.dma_start(g7[:], g7_d.ap()[:, :])
            negT7 = cpool.tile([128, 1], F32, tag="negT7")
            nc.sync.dma_start(negT7[:], negT7_d.ap()[:, :])
            a_s = cpool.tile([128, N_CLASSES], F16, tag="A")
            nc.sync.dma_start(a_s[:], a_d.ap()[:, :])
            b_s = cpool.tile([128, N_CLASSES], F16, tag="B")
            nc.sync.dma_start(b_s[:], b_d.ap()[:, :])

            # P7 staging in DRAM, row order q = (s, p, g)
            p7dram = dpool.tile([R, 128], F16, tag="p7dram")
            p7dram_v = p7dram[:].rearrange(
                "(s p g) j -> s p g j", s=N_SUPER, p=TILE, g=G_TILES)

            pt_out = pt_pool.tile([128, 8 * FIN_SUB], F32, tag="ptout")

            fin_state = {}

            for s in range(N_SUPER):
                act06 = act_pool.tile([TILE, G_TILES, 128], F16, tag="act06")
                # --- z06 matmuls + sigmoid, 8 tiles per PSUM buffer ---
                for pb in range(G_TILES // PSUM_TILES):
                    z06 = zpsum.tile([TILE, PSUM_TILES * 128], F32, tag="z06",
                                     name="z06")
                    for i in range(PSUM_TILES):
                        t = s * G_TILES + pb * PSUM_TILES + i
                        nc.tensor.matmul(
                            z06[:, bass.ts(i, 128)],
                            xT[:, bass.ts(t, TILE)], g06[:],
                            start=True, stop=True,
                        )
                    nc.scalar.activation(
                        act06[:, bass.ts(pb, PSUM_TILES), :], z06[:], SIGMOID)
                # --- tree DP levels 1..6 (batch-major, grouped) ---
                prev = act06[:, :, 0:2]
                for d in range(1, 7):
                    n = 1 << d
                    if d < 6:
                        cur = tree_pool.tile([TILE, G_TILES, 2 * n], F16,
                                             tag=f"P{d + 1}", name=f"P{d + 1}")
                    else:
                        cur = p7_pool.tile([TILE, G_TILES, 128], F16,
                                           tag="P7", name="P7")
                    nc.vector.tensor_mul(
                        cur[:, :, n:2 * n], prev[:], act06[:, :, n:2 * n])
                    nc.vector.tensor_sub(
                        cur[:, :, 0:n], prev[:], cur[:, :, n:2 * n])
                    prev = cur[:, :, :]
                nc.sync.dma_start(p7dram_v[s], prev[:])

                # --- transpose this supertile back, node-major ---
                p7t = p7t_pool.tile([128, SUPER_ROWS], F16, tag="P7T",
                                    name="p7t")
                nc.sync.dma_start_transpose(
                    p7t[:], p7dram[bass.ts(s, SUPER_ROWS), :])

                # node-major gates for this supertile's chunks
                sig7s = {}
                for cc in range(SUPER_ROWS // NM_CHUNK):
                    ch = s * (SUPER_ROWS // NM_CHUNK) + cc
                    z7t = z7psum.tile([128, NM_CHUNK], F32, tag="z7T",
                                      name="z7t")
                    for hf in range(NM_CHUNK // FIN_SUB):
                        rhs = xq[:, ch * NM_CHUNK + hf * FIN_SUB:
                                 ch * NM_CHUNK + (hf + 1) * FIN_SUB]
                        nc.tensor.matmul(z7t[:, bass.ts(hf, FIN_SUB)],
                                         g7[:], rhs, start=True, stop=True)
                    sig7 = sig_pool.tile([128, NM_CHUNK], F16,
                                         tag=f"sig{ch % 8}",
                                         name=f"sig{ch % 8}")
                    nc.scalar.activation(sig7[:], z7t[:], SIGMOID,
                                         bias=negT7[:])
                    sig7s[ch] = sig7

                for cc in range(SUPER_ROWS // NM_CHUNK):
                    ch = s * (SUPER_ROWS // NM_CHUNK) + cc
                    r7t = nm_pool.tile([128, NM_CHUNK], F16, tag="R7T",
                                       name="r7t")
                    p7t_sl = p7t[:, bass.ts(cc, NM_CHUNK)]
                    nc.vector.tensor_mul(r7t[:], p7t_sl, sig7s[ch][:])
                    # finals: col-group cg cycles 0..3; 4 subs share a psum
                    for half in range(NM_CHUNK // FIN_SUB):
                        u = ch * (NM_CHUNK // FIN_SUB) + half  # 0..31
                        k, cg = divmod(u, 4)
                        if cg == 0:
                            fin_state["fp"] = fpsum.tile(
                                [128, FIN_SUB], F32, tag="fin", name="fin")
                        fp = fin_state["fp"]
                        out_sl = fp[32 * cg:32 * cg + N_CLASSES, :]
                        rhs_p = p7t[:, cc * NM_CHUNK + half * FIN_SUB:
                                    cc * NM_CHUNK + (half + 1) * FIN_SUB]
                        rhs_r = r7t[:, bass.ts(half, FIN_SUB)]
                        nc.tensor.matmul(out_sl, a_s[:], rhs_p,
                                         start=True, stop=False,
                                         tile_position=(0, 32 * cg))
                        nc.tensor.matmul(out_sl, b_s[:], rhs_r,
                                         start=False, stop=True,
                                         tile_position=(0, 32 * cg))
                        if cg == 3:
                            nc.vector.tensor_copy(
                                pt_out[:, bass.ts(k, FIN_SUB)],
                                fp[:, 0:FIN_SUB])

            # --- output DMA: 4 strided DMAs, one per col-group ---
            # pt_out[32*cg + c, k*512 + scol] = pT[c, q], q = (4k+cg)*512+scol
            pt_v = pt_d.ap().rearrange("c (k cg scol) -> c k cg scol",
                                       k=8, cg=4, scol=FIN_SUB)
            for cg in range(4):
                src = pt_out[32 * cg:32 * cg + N_CLASSES, :].rearrange(
                    "c (k scol) -> c k scol", k=8, scol=FIN_SUB)
                nc.sync.dma_start(pt_v[:, :, cg, :], src)

    nc.compile()
    return nc


def _get_compiled():
    global _COMPILED
    if _COMPILED is None:
        _COMPILED = _build_program()
    return _COMPILED


def kernel(x, thresholds, feats, leaf_class, _trace=False):
    x = np.asarray(x, dtype=np.float32)
    thresholds = np.asarray(thresholds, dtype=np.float32)
    feats = np.asarray(feats, dtype=np.int32)
    leaf_class = np.asarray(leaf_class, dtype=np.int32)
    assert x.shape == (BATCH, N_FEAT)

    G06, G7, negT7, A, Bm = _host_prep(thresholds, feats, leaf_class)
    f16 = np.float16

    x_ext_T = np.empty((KDIM, BATCH), dtype=f16)
    x_ext_T[:N_FEAT, :] = x.T.astype(f16)
    x_ext_T[N_FEAT, :] = 1.0

    in_maps = []
    for c in range(N_CORES):
        sl = slice(c * R, (c + 1) * R)
        xt_c = np.ascontiguousarray(x_ext_T[:, sl])
        # q-order: q = s*4096 + p*32 + g  <->  b = s*4096 + g*128 + p
        xq_c = np.ascontiguousarray(
            xt_c[:N_FEAT].reshape(N_FEAT, N_SUPER, G_TILES, TILE)
            .transpose(0, 1, 3, 2).reshape(N_FEAT, R))
        in_maps.append({
            "xT": xt_c,
            "xQ": xq_c,
            "G06": G06.astype(f16),
            "G7": G7.astype(f16),
            "negT7": negT7,
            "A": A.astype(f16),
            "B": Bm.astype(f16),
        })

    nc = _get_compiled()
    res = bass_utils.run_bass_kernel_spmd(
        nc, in_maps, core_ids=list(range(N_CORES)),
        trace=_trace, trace_cores=[0] if _trace else None,
    )

    out = np.empty((BATCH, N_CLASSES), dtype=np.float32)
    for c in range(N_CORES):
        sl = slice(c * R, (c + 1) * R)
        pt = res.results[c]["pT"]  # [10, R], cols in q=(s,p,g) order
        # q = s*4096 + p*32 + g ; b = s*4096 + g*128 + p
        pt_b = pt.reshape(N_CLASSES, N_SUPER, TILE, G_TILES) \
                 .transpose(0, 1, 3, 2).reshape(N_CLASSES, R)
        out[sl] = pt_b.T
    if _trace:
        kernel._last_results = res
    return out


# revision 25
# speedup vs baseline: 1.9699x; 1.2330x over previous
"""Soft decision-tree forward kernel for Trainium2 (8 NeuronCores, SPMD).

Per core (16384 rows), fp16 data path, f32 accumulation:
  1. z06 = [x|1]^T-tile @ G06          (PE, per 128-row tile, K=33, N=128)
  2. act06 = sigmoid(z06)              (ACT, one op per 8 tiles, fp16 out)
  3. tree DP levels 1..6, batch-major  (DVE, multi-AP ops over 32-tile groups)
  4. P7 -> DRAM -> xbar-transpose back (DMA) giving node-major P7T [128, B]
  5. z7T = G7 @ xq, sig7T = sigmoid(z7T - T7)   (PE K=32 + ACT bias, early)
  6. R7T = P7T * sig7T                 (DVE, node-major)
  7. pT = A^T@P7T + B^T@R7T            (PE, col-tiled 4x, PSUM accumulate)
  8. pT -> SBUF -> DRAM [10, 16384]    (host un-permutes columns)

Node layout: level d's column k <-> heap node bitrev_d(k) (concat child
placement). All reorderings are baked into G06 / G7 / A / B host-side.
P7 DRAM row order is q = (s, p, g): s = supertile, p = row-in-tile,
g = tile-in-supertile; batch b = s*4096 + g*128 + p. xQ input carries x
columns pre-permuted to q order so node-major matmul reads are contiguous.
"""

import sys

sys.path.insert(0, "/opt/trn_rl_repo")

import numpy as np

import concourse.bacc as bacc
import concourse.bass as bass
import concourse.mybir as mybir
import concourse.tile as tile
from concourse import bass_utils

# ---- problem constants (hardcoded per contract) ----
BATCH = 131072
N_FEAT = 32
N_CLASSES = 10
N_CORES = 8
R = BATCH // N_CORES          # 16384 rows per core
TILE = 128
N_TILES = R // TILE           # 128
G_TILES = 32                  # tiles per supertile (DVE grouping)
N_SUPER = N_TILES // G_TILES  # 4
SUPER_ROWS = G_TILES * TILE   # 4096
KDIM = N_FEAT + 1             # 33
PSUM_TILES = 8                # z06 tiles per PSUM buffer -> ACT op of N=1024
NM_CHUNK = 1024               # node-major batch chunk (z7T / sig7T / R7T)
N_CHUNKS = R // NM_CHUNK      # 16
FIN_SUB = 512                 # final matmul sub-chunk (one PSUM bank)

F32 = mybir.dt.float32
F16 = mybir.dt.float16
SIGMOID = mybir.ActivationFunctionType.Sigmoid

_COMPILED = None


def _bitrev(k, bits):
    r = 0
    for _ in range(bits):
        r = (r << 1) | (k & 1)
        k >>= 1
    return r


def _host_prep(thresholds, feats, leaf_class):
    """G06 [33,128], G7 [32,128], negT7 [128,1], A/B [128,10] (device layout)."""
    G06 = np.zeros((KDIM, 128), dtype=np.float32)
    f0, t0 = int(feats[0]), float(thresholds[0])
    G06[f0, 0] = -1.0
    G06[N_FEAT, 0] = +t0
    G06[f0, 1] = +1.0
    G06[N_FEAT, 1] = -t0
    for d in range(1, 7):
        n = 1 << d
        start = n - 1
        for k in range(n):
            j = _bitrev(k, d)
            G06[int(feats[start + j]), n + k] = 1.0
            G06[N_FEAT, n + k] = -float(thresholds[start + j])
    G7 = np.zeros((N_FEAT, 128), dtype=np.float32)
    negT7 = np.zeros((128, 1), dtype=np.float32)
    start7 = 127
    for k in range(128):
        j = _bitrev(k, 7)
        G7[int(feats[start7 + j]), k] = 1.0
        negT7[k, 0] = -float(thresholds[start7 + j])
    Lc = np.empty(128, dtype=np.int64)
    Rc = np.empty(128, dtype=np.int64)
    for k in range(128):
        j7 = _bitrev(k, 7)
        Lc[k] = leaf_class[2 * j7]
        Rc[k] = leaf_class[2 * j7 + 1]
    A = np.zeros((128, N_CLASSES), dtype=np.float32)
    Bm = np.zeros((128, N_CLASSES), dtype=np.float32)
    A[np.arange(128), Lc] = 1.0
    Bm[np.arange(128), Rc] += 1.0
    Bm[np.arange(128), Lc] -= 1.0
    return G06, G7, negT7, A, Bm


def _build_program():
    nc = bacc.Bacc("TRN2", target_bir_lowering=False, debug=False,
                   num_devices=N_CORES)

    xT_d = nc.dram_tensor("xT", [KDIM, R], F16, kind="ExternalInput")
    xq_d = nc.dram_tensor("xQ", [N_FEAT, R], F16, kind="ExternalInput")
    g06_d = nc.dram_tensor("G06", [KDIM, 128], F16, kind="ExternalInput")
    g7_d = nc.dram_tensor("G7", [N_FEAT, 128], F16, kind="ExternalInput")
    negT7_d = nc.dram_tensor("negT7", [128, 1], F32, kind="ExternalInput")
    a_d = nc.dram_tensor("A", [128, N_CLASSES], F16, kind="ExternalInput")
    b_d = nc.dram_tensor("B", [128, N_CLASSES], F16, kind="ExternalInput")
    pt_d = nc.dram_tensor("pT", [N_CLASSES, R], F16, kind="ExternalOutput")

    with tile.TileContext(nc) as tc:
        with (
            tc.tile_pool(name="const", bufs=1) as cpool,
            tc.tile_pool(name="dram", bufs=1, space="DRAM") as dpool,
            tc.tile_pool(name="act06", bufs=2) as act_pool,
            tc.tile_pool(name="ptree", bufs=1) as tree_pool,
            tc.tile_pool(name="p7", bufs=2) as p7_pool,
            tc.tile_pool(name="p7t", bufs=2) as p7t_pool,
            tc.tile_pool(name="sig", bufs=1) as sig_pool,
            tc.tile_pool(name="nm", bufs=2) as nm_pool,
            tc.tile_pool(name="ptout", bufs=1) as pt_pool,
            tc.tile_pool(name="zpsum", bufs=2, space="PSUM") as zpsum,
            tc.tile_pool(name="z7psum", bufs=1, space="PSUM") as z7psum,
            tc.tile_pool(name="fpsum", bufs=2, space="PSUM") as fpsum,
        ):
            g06 = cpool.tile([KDIM, 128], F16, tag="G06")
            nc.sync.dma_start(g06[:], g06_d.ap()[:, :])
            g7 = cpool.tile([N_FEAT, 128], F16, tag="G7")
            nc.sync.dma_start(g7[:], g7_d.ap()[:, :])
            negT7 = cpool.tile([128, 1], F32, tag="negT7")
            nc.sync.dma_start(negT7[:], negT7_d.ap()[:, :])
            a_s = cpool.tile([128, N_CLASSES], F16, tag="A")
            nc.sync.dma_start(a_s[:], a_d.ap()[:, :])
            b_s = cpool.tile([128, N_CLASSES], F16, tag="B")
            nc.sync.dma_start(b_s[:], b_d.ap()[:, :])
            # x loads split per supertile so compute starts after chunk 0
            xt_parts, xq_parts = [], []
            for s in range(N_SUPER):
                xt_p = cpool.tile([KDIM, SUPER_ROWS], F16, tag=f"xTp{s}",
                                  name=f"xTp{s}")
                nc.sync.dma_start(
                    xt_p[:], xT_d.ap()[:, bass.ts(s, SUPER_ROWS)])
                xt_parts.append(xt_p)
                xq_p = cpool.tile([N_FEAT, SUPER_ROWS], F16, tag=f"xQp{s}",
                                  name=f"xQp{s}")
                nc.sync.dma_start(
                    xq_p[:], xq_d.ap()[:, bass.ts(s, SUPER_ROWS)])
                xq_parts.append(xq_p)

            # P7 staging in DRAM, row order q = (s, p, g)
            p7dram = dpool.tile([R, 128], F16, tag="p7dram")
            p7dram_v = p7dram[:].rearrange(
                "(s p g) j -> s p g j", s=N_SUPER, p=TILE, g=G_TILES)

            pt_out = pt_pool.tile([128, 8 * FIN_SUB], F16, tag="ptout")

            fin_state = {}

            def emit_finals(s, p7t, sig7s):
                """R7T + final matmuls + PSUM drain for supertile s."""
                for cc in range(SUPER_ROWS // NM_CHUNK):
                    ch = s * (SUPER_ROWS // NM_CHUNK) + cc
                    r7t = nm_pool.tile([128, NM_CHUNK], F16, tag="R7T",
                                       name="r7t")
                    p7t_sl = p7t[:, bass.ts(cc, NM_CHUNK)]
                    nc.vector.tensor_mul(r7t[:], p7t_sl, sig7s[cc][:])
                    # finals: col-group cg cycles 0..3; 4 subs share a psum
                    for half in range(NM_CHUNK // FIN_SUB):
                        u = ch * (NM_CHUNK // FIN_SUB) + half  # 0..31
                        k, cg = divmod(u, 4)
                        if cg == 0:
                            fin_state["fp"] = fpsum.tile(
                                [128, FIN_SUB], F32, tag="fin", name="fin")
                        fp = fin_state["fp"]
                        out_sl = fp[32 * cg:32 * cg + N_CLASSES, :]
                        rhs_p = p7t[:, cc * NM_CHUNK + half * FIN_SUB:
                                    cc * NM_CHUNK + (half + 1) * FIN_SUB]
                        rhs_r = r7t[:, bass.ts(half, FIN_SUB)]
                        nc.tensor.matmul(out_sl, a_s[:], rhs_p,
                                         start=True, stop=False,
                                         tile_position=(0, 32 * cg))
                        nc.tensor.matmul(out_sl, b_s[:], rhs_r,
                                         start=False, stop=True,
                                         tile_position=(0, 32 * cg))
                        if cg == 3:
                            nc.vector.tensor_copy(
                                pt_out[:, bass.ts(k, FIN_SUB)],
                                fp[:, 0:FIN_SUB])

            pending = None
            for s in range(N_SUPER):
                act06 = act_pool.tile([TILE, G_TILES, 128], F16, tag="act06")
                # --- z06 matmuls + sigmoid, 8 tiles per PSUM buffer ---
                for pb in range(G_TILES // PSUM_TILES):
                    z06 = zpsum.tile([TILE, PSUM_TILES * 128], F32, tag="z06",
                                     name="z06")
                    for i in range(PSUM_TILES):
                        lt = pb * PSUM_TILES + i  # tile within supertile
                        nc.tensor.matmul(
                            z06[:, bass.ts(i, 128)],
                            xt_parts[s][:, bass.ts(lt, TILE)], g06[:],
                            start=True, stop=True,
                        )
                    nc.scalar.activation(
                        act06[:, bass.ts(pb, PSUM_TILES), :], z06[:], SIGMOID)
                # --- tree DP levels 1..6 (batch-major, grouped) ---
                prev = act06[:, :, 0:2]
                for d in range(1, 7):
                    n = 1 << d
                    if d < 6:
                        cur = tree_pool.tile([TILE, G_TILES, 2 * n], F16,
                                             tag=f"P{d + 1}", name=f"P{d + 1}")
                    else:
                        cur = p7_pool.tile([TILE, G_TILES, 128], F16,
                                           tag="P7", name="P7")
                    nc.vector.tensor_mul(
                        cur[:, :, n:2 * n], prev[:], act06[:, :, n:2 * n])
                    nc.vector.tensor_sub(
                        cur[:, :, 0:n], prev[:], cur[:, :, n:2 * n])
                    prev = cur[:, :, :]
                nc.sync.dma_start(p7dram_v[s], prev[:])

                # --- transpose this supertile back, node-major ---
                p7t = p7t_pool.tile([128, SUPER_ROWS], F16, tag="P7T",
                                    name="p7t")
                nc.sync.dma_start_transpose(
                    p7t[:], p7dram[bass.ts(s, SUPER_ROWS), :])

                # node-major gates for this supertile's chunks
                sig7s = []
                for cc in range(SUPER_ROWS // NM_CHUNK):
                    ch = s * (SUPER_ROWS // NM_CHUNK) + cc
                    z7t = z7psum.tile([128, NM_CHUNK], F32, tag="z7T",
                                      name="z7t")
                    for hf in range(NM_CHUNK // FIN_SUB):
                        lo = cc * NM_CHUNK + hf * FIN_SUB
                        rhs = xq_parts[s][:, lo:lo + FIN_SUB]
                        nc.tensor.matmul(z7t[:, bass.ts(hf, FIN_SUB)],
                                         g7[:], rhs, start=True, stop=True)
                    sig7 = sig_pool.tile([128, NM_CHUNK], F16,
                                         tag=f"sig{ch % 8}",
                                         name=f"sig{ch % 8}")
                    nc.scalar.activation(sig7[:], z7t[:], SIGMOID,
                                         bias=negT7[:])
                    sig7s.append(sig7)

                # finals for the PREVIOUS supertile (keeps PE from stalling
                # on this supertile's transpose before starting the next)
                if pending is not None:
                    emit_finals(*pending)
                pending = (s, p7t, sig7s)
            emit_finals(*pending)

            # --- output DMA: 4 strided DMAs, one per col-group ---
            # pt_out[32*cg + c, k*512 + scol] = pT[c, q], q = (4k+cg)*512+scol
            pt_v = pt_d.ap().rearrange("c (k cg scol) -> c k cg scol",
                                       k=8, cg=4, scol=FIN_SUB)
            for cg in range(4):
                src = pt_out[32 * cg:32 * cg + N_CLASSES, :].rearrange(
                    "c (k scol) -> c k scol", k=8, scol=FIN_SUB)
                nc.sync.dma_start(pt_v[:, :, cg, :], src)

    nc.compile()
    return nc


def _get_compiled():
    global _COMPILED
    if _COMPILED is None:
        _COMPILED = _build_program()
    return _COMPILED


def kernel(x, thresholds, feats, leaf_class, _trace=False):
    x = np.asarray(x, dtype=np.float32)
    thresholds = np.asarray(thresholds, dtype=np.float32)
    feats = np.asarray(feats, dtype=np.int32)
    leaf_class = np.asarray(leaf_class, dtype=np.int32)
    assert x.shape == (BATCH, N_FEAT)

    G06, G7, negT7, A, Bm = _host_prep(thresholds, feats, leaf_class)
    f16 = np.float16

    x_ext_T = np.empty((KDIM, BATCH), dtype=f16)
    x_ext_T[:N_FEAT, :] = x.T.astype(f16)
    x_ext_T[N_FEAT, :] = 1.0

    in_maps = []
    for c in range(N_CORES):
        sl = slice(c * R, (c + 1) * R)
        xt_c = np.ascontiguousarray(x_ext_T[:, sl])
        # q-order: q = s*4096 + p*32 + g  <->  b = s*4096 + g*128 + p
        xq_c = np.ascontiguousarray(
            xt_c[:N_FEAT].reshape(N_FEAT, N_SUPER, G_TILES, TILE)
            .transpose(0, 1, 3, 2).reshape(N_FEAT, R))
        in_maps.append({
            "xT": xt_c,
            "xQ": xq_c,
            "G06": G06.astype(f16),
            "G7": G7.astype(f16),
            "negT7": negT7,
            "A": A.astype(f16),
            "B": Bm.astype(f16),
        })

    nc = _get_compiled()
    res = bass_utils.run_bass_kernel_spmd(
        nc, in_maps, core_ids=list(range(N_CORES)),
        trace=_trace, trace_cores=[0] if _trace else None,
    )

    out = np.empty((BATCH, N_CLASSES), dtype=np.float32)
    for c in range(N_CORES):
        sl = slice(c * R, (c + 1) * R)
        pt = res.results[c]["pT"].astype(np.float32)  # [10, R], q-order cols
        # q = s*4096 + p*32 + g ; b = s*4096 + g*128 + p
        pt_b = pt.reshape(N_CLASSES, N_SUPER, TILE, G_TILES) \
                 .transpose(0, 1, 3, 2).reshape(N_CLASSES, R)
        out[sl] = pt_b.T
    if _trace:
        kernel._last_results = res
    return out
